# revision 18
# baseline (speedup 1.0000x reference)
"""Trainium2 Bass kernel: grouped MoE expert MLP (nn_ExpertGroup).

Strategy: expert parallelism across 8 NeuronCores. Tokens are sorted by
expert; core e runs expert e's two GEMMs:
    h = relu(x_e @ w_up[e].T) ** 2      (bf16, like the CUDA reference)
    y = h @ w_down[e].T
The host does the (free) token scatter/gather, the bf16 casts, and the
layout pre-packing so device DMAs read fat contiguous per-partition runs.

Host-packed HBM layouts (per core, cap = padded local token count):
    xp  [128, n_d*cap]     xp[pi, d*cap+t]         = x_e.T[d*128+pi, t]
    wup [128, n_j*n_d*128] wup[pi,(j*n_d+d)*128+c] = w_up[e].T[d*128+pi, j*128+c]
    wdp [128, n_j*D]       wdp[pi, j*D+i]          = w_down[e].T[j*128+pi, i]

Schedule (v5, driven by NTFF traces; the measured exec window runs from
the kernel-body start at ~6us to the LAST teardown instruction, so both
the DMA critical path at the head and the queue-drain tail count):
  - Each dma_start costs a flat ~0.6us trigger on the Sync queue, so the
    input is fetched with few triggers, ordered exactly as consumed.
  - GEMM1's first token chunk is 256 wide: the critical first bundle is
    only wu_j0 (0.25MB) + x[:, 0:256] (0.5MB), so real matmuls start
    ~3.5us earlier than with a 512-wide first chunk. w_up then streams
    per-j (0.25MB chunks): at 256-wide groups the PE consumes weights at
    ~290GB/s, just under the DMA rate, so per-j sems stay ahead.
  - PE warm-up dummies (no DMA deps) bridge preamble-end -> first data
    and keep the HAM clock ramp going (a PE gap costs ~2x: idle plus
    ~3us of 1.2GHz re-ramp, visible as 379ns matmuls in the trace).
  - relu runs on the Scalar (ACT) engine, the square on Vector: one op
    per engine per tile instead of two DVE ops.
  - PSUM start/stop accumulation is bank-granular: every accumulation
    group owns a whole [128, 512] PSUM tile (narrow groups just use a
    prefix of it). Never split one bank into independent regions.
  - Two tile pools (SBUF + PSUM) with per-tag bufs instead of five: the
    pool-exit barrier chain at the end of the measured window shrinks.
  - Output: one [128, 1024] DMA per token tile (both ic halves cast into
    one SBUF tile); the last tile runs ic0 + two 256-col groups in two
    PSUM banks so the first half's cast+DMA overlaps the second half's
    matmuls -> shorter serial tail.

Built on bacc.Bacc (not raw Bass): Bacc.compile() legalizes semaphore
waits to the TRN2 limit of one wait per instruction.
"""

import numpy as np
import ml_dtypes

import concourse.bass as bass
import concourse.mybir as mybir
import concourse.tile as tile
from concourse import bacc
from concourse.bass_utils import run_bass_kernel_spmd

T, D, H, E = 8192, 1024, 2048, 8
P = 128
N_CORES = 8
FD = 512   # matmul moving free dim (one PSUM bank of fp32)
C0 = 256   # width of the first (DMA-latency-critical) token chunk
# Dummy matmuls bridging the preamble->first-data window. The first input
# bundle lands 12.4-13.0us across runs (fixed DMA ring-start + sem latency,
# not descriptor-bound). Undershooting idles the PE and costs a ~1.4us HAM
# clock re-ramp (8 matmuls at 379ns instead of 216); overshooting only
# delays the stream by the overshoot. So size the warmup to end just PAST
# the p90 arrival (~13.4us): 70 dummies at ~56-107ns each.
WARMUP = 70


def _ensure_axon_ntff_hook():
    """The container's `antenv` stub lacks `axon_hooks`; if BASS_TRACE=1 is
    set, run_bass_kernel_spmd would crash importing it. Recreate the tiny
    registry and register the ctypes NTFF hook so tracing works (and never
    let this best-effort setup break the kernel)."""
    try:
        import antenv.axon_hooks  # noqa: F401
        return
    except ImportError:
        pass
    try:
        import sys
        import types

        import antenv
        from trn_agent_boot.trn_boot import _ntff_profile_via_ctypes

        mod = types.ModuleType("antenv.axon_hooks")
        mod._hook = _ntff_profile_via_ctypes("/opt/axon/libaxon_pjrt.so")
        mod.set_axon_ntff_profile_hook = lambda h: setattr(mod, "_hook", h)
        mod.get_axon_ntff_profile_hook = lambda: mod._hook
        sys.modules["antenv.axon_hooks"] = mod
        antenv.axon_hooks = mod
    except Exception:
        pass


_ensure_axon_ntff_hook()

_PROGRAM_CACHE: dict[int, "bass.Bass"] = {}
LAST_RESULT = None  # BassKernelResults of the most recent run (for harness use)


def _chunks(cap):
    """GEMM1 token chunks: first 512 split as 256+256 so the critical
    first DMA bundle is small; later chunks are full width."""
    n_tc = cap // FD
    return [(0, C0), (C0, FD)] + [(k * FD, (k + 1) * FD) for k in range(1, n_tc)]


def _build_program(cap: int) -> "bass.Bass":
    assert cap % FD == 0
    n_d = D // P       # 8  contraction tiles of GEMM1
    n_j = H // P       # 16 H partition tiles
    n_tc = cap // FD   # 512-wide token chunks
    n_t = cap // P     # token partition tiles (GEMM2 output)
    bf16 = mybir.dt.bfloat16
    f32 = mybir.dt.float32
    relu_fn = mybir.ActivationFunctionType.Relu

    nc = bacc.Bacc("TRN2", debug=False, num_devices=N_CORES)
    xp = nc.dram_tensor("xp", [P, n_d * cap], bf16, kind="ExternalInput")
    wup = nc.dram_tensor("wup", [P, n_j * n_d * P], bf16, kind="ExternalInput")
    wdp = nc.dram_tensor("wdp", [P, n_j * D], bf16, kind="ExternalInput")
    y = nc.dram_tensor("y", [cap, D], bf16, kind="ExternalOutput")

    wup4 = wup[:].rearrange("p (j d c) -> p j d c", j=n_j, d=n_d)
    wdp3 = wdp[:].rearrange("p (j f) -> p j f", j=n_j)          # [128, 16, D]
    y3 = y[:].rearrange("(po pi) f -> pi po f", pi=P)           # [128, n_t, D]

    chunks = _chunks(cap)

    with tile.TileContext(nc) as tc:
        with (
            tc.tile_pool(name="sb", bufs=1) as sb,
            tc.tile_pool(name="ps", bufs=7, space="PSUM") as psp,
        ):
            # x lives chunk-major in one flat SBUF region (matching the
            # host xp packing) so every x DMA is ONE contiguous run per
            # partition: 128 fat descriptors instead of 1024 small ones
            # (the strided form is descriptor-rate-bound, ~5us for 0.5MB).
            x_sb = sb.tile([P, n_d * cap], bf16)
            wuT_sb = sb.tile([P, n_j, n_d, P], bf16)
            wdT_sb = sb.tile([P, n_j, D], bf16)
            hsq_sb = sb.tile([P, n_j, cap], bf16)

            def xs(lo, hi, d):
                w = hi - lo
                return x_sb[:, n_d * lo + d * w: n_d * lo + (d + 1) * w]

            # Input DMAs, emitted first so the Sync queue leads with the
            # triggers (flat ~0.6us each), ordered exactly as consumed.
            # NOTE: do NOT prepend a tiny (sub-512B-per-partition) warm-up
            # DMA — runs with one consistently executed with the whole PE
            # clock ladder capped ~20% low (153us instead of 128us).
            lo0, hi0 = chunks[0]
            nc.sync.dma_start(out=wuT_sb[:, 0, :, :], in_=wup4[:, 0, :, :])
            nc.sync.dma_start(
                out=x_sb[:, n_d * lo0:n_d * hi0], in_=xp[:, n_d * lo0:n_d * hi0]
            )
            for j in range(1, n_j):
                nc.sync.dma_start(
                    out=wuT_sb[:, j, :, :], in_=wup4[:, j, :, :]
                )
            for lo, hi in chunks[1:]:
                nc.sync.dma_start(
                    out=x_sb[:, n_d * lo:n_d * hi], in_=xp[:, n_d * lo:n_d * hi]
                )
            for q in range(2):
                nc.sync.dma_start(
                    out=wdT_sb[:, q * 8:(q + 1) * 8, :],
                    in_=wdp3[:, q * 8:(q + 1) * 8, :],
                )

            # PE warm-up: dummy matmuls with no DMA dependencies bridge the
            # window from preamble end to first-data arrival and start the
            # HAM clock ramp early.
            warm = sb.tile([P, P], bf16)
            nc.vector.memset(warm[:], 0.0)
            wps = psp.tile([P, P], f32, tag="warm", bufs=1)
            for _ in range(WARMUP):
                nc.tensor.matmul(wps, warm[:], warm[:], start=True, stop=True)

            # GEMM1 + relu^2: hsq[j, t] = relu(h)^2 in [H, T] layout.
            for lo, hi in chunks:
                w = hi - lo
                for j in range(n_j):
                    ps = psp.tile([P, FD], f32, tag="ps")
                    for d in range(n_d):
                        nc.tensor.matmul(
                            ps[:, 0:w],
                            wuT_sb[:, j, d, :],
                            xs(lo, hi, d),
                            start=(d == 0),
                            stop=(d == n_d - 1),
                        )
                    hr = sb.tile([P, FD], bf16, tag="hr", bufs=4)
                    nc.scalar.activation(hr[:, 0:w], ps[:, 0:w], relu_fn)
                    nc.vector.tensor_mul(
                        out=hsq_sb[:, j, lo:hi], in0=hr[:, 0:w], in1=hr[:, 0:w]
                    )

            # GEMM2: y[t, i] = sum_j hsq[j, t].T @ wdT[j, i]
            n_ic = D // FD
            for t in range(n_t):
                yt = sb.tile([P, D], bf16, tag="yt", bufs=4)
                if t < n_t - 1:
                    # Both ic halves cast into one SBUF row tile -> one
                    # [128, 1024] out DMA per token tile.
                    for ic in range(n_ic):
                        ps = psp.tile([P, FD], f32, tag="ps")
                        for j in range(n_j):
                            nc.tensor.matmul(
                                ps,
                                hsq_sb[:, j, t * P:(t + 1) * P],
                                wdT_sb[:, j, ic * FD:(ic + 1) * FD],
                                start=(j == 0),
                                stop=(j == n_j - 1),
                            )
                        nc.vector.tensor_copy(
                            out=yt[:, ic * FD:(ic + 1) * FD], in_=ps
                        )
                    nc.sync.dma_start(out=y3[:, t, :], in_=yt)
                else:
                    ps = psp.tile([P, FD], f32, tag="ps")
                    for j in range(n_j):
                        nc.tensor.matmul(
                            ps,
                            hsq_sb[:, j, t * P:(t + 1) * P],
                            wdT_sb[:, j, 0:FD],
                            start=(j == 0),
                            stop=(j == n_j - 1),
                        )
                    nc.vector.tensor_copy(out=yt[:, 0:FD], in_=ps)
                    nc.sync.dma_start(out=y3[:, t, 0:FD], in_=yt[:, 0:FD])
                    # Final 512 cols: two 256-col groups in two PSUM banks;
                    # the first half's cast+DMA overlaps the second half's
                    # matmuls -> shorter serial tail.
                    for h in range(2):
                        hw = FD // 2
                        osl = slice(FD + h * hw, FD + (h + 1) * hw)
                        ph = psp.tile([P, FD], f32, tag="ps")
                        for j in range(n_j):
                            nc.tensor.matmul(
                                ph[:, 0:hw],
                                hsq_sb[:, j, t * P:(t + 1) * P],
                                wdT_sb[:, j, osl],
                                start=(j == 0),
                                stop=(j == n_j - 1),
                            )
                        nc.vector.tensor_copy(out=yt[:, osl], in_=ph[:, 0:hw])
                        nc.sync.dma_start(out=y3[:, t, osl], in_=yt[:, osl])

    nc.compile()
    return nc


def _get_program(cap: int) -> "bass.Bass":
    nc = _PROGRAM_CACHE.get(cap)
    if nc is None:
        nc = _build_program(cap)
        _PROGRAM_CACHE[cap] = nc
    return nc


CAP = 1024  # tokens per core per round (the uniform T/E split = one round)


def _pack_x(xs_T):
    """x_e.T (D, cap) -> xp [128, n_d*cap] chunk-major packed: for each
    token chunk [lo,hi), block[pi, n_d*lo + d*(hi-lo) + tl] =
    xs_T[d*128+pi, lo+tl]."""
    Dd, cap = xs_T.shape
    xt = xs_T.reshape(D // P, P, cap)
    blocks = [
        xt[:, :, lo:hi].transpose(1, 0, 2).reshape(P, (D // P) * (hi - lo))
        for lo, hi in _chunks(cap)
    ]
    return np.ascontiguousarray(np.concatenate(blocks, axis=1))


def _pack_wu(wuT):
    """w_up[e].T (D, H) -> wup [128, n_j*n_d*128]."""
    return np.ascontiguousarray(
        wuT.reshape(D // P, P, H // P, P)
        .transpose(1, 2, 0, 3)
        .reshape(P, (H // P) * (D // P) * P)
    )


def _pack_wd(wdT):
    """w_down[e].T (H, D) -> wdp [128, n_j*D]."""
    return np.ascontiguousarray(
        wdT.reshape(H // P, P, D).transpose(1, 0, 2).reshape(P, (H // P) * D)
    )


def kernel(x, num_tokens_per_expert, w_up, w_down, _trace=False):
    global LAST_RESULT
    bf = ml_dtypes.bfloat16
    x = np.asarray(x)
    counts = np.asarray(num_tokens_per_expert).astype(np.int64)
    w_up = np.asarray(w_up)
    w_down = np.asarray(w_down)
    n_tok = x.shape[0]
    assert counts.shape == (E,) and int(counts.sum()) == n_tok
    offsets = np.zeros(E, dtype=np.int64)
    offsets[1:] = np.cumsum(counts)[:-1]

    nc = _get_program(CAP)

    # Work list: split each expert's contiguous token segment into slots of
    # <= CAP tokens; process 8 slots per SPMD round. The uniform T/E = 1024
    # split is exactly one round of 8 slots.
    slots = []
    for e in range(E):
        cnt, off = int(counts[e]), int(offsets[e])
        for s in range(0, cnt, CAP):
            slots.append((e, off + s, min(CAP, cnt - s)))

    wu_cache = {}
    wd_cache = {}

    def expert_weights(e):
        if e not in wu_cache:
            wu_cache[e] = _pack_wu(w_up[e].astype(bf).T)
            wd_cache[e] = _pack_wd(w_down[e].astype(bf).T)
        return wu_cache[e], wd_cache[e]

    out = np.zeros((n_tok, D), dtype=x.dtype)
    zero_map = None
    for r0 in range(0, len(slots), N_CORES):
        round_slots = slots[r0:r0 + N_CORES]
        in_maps = []
        for e, off, cnt in round_slots:
            xs = np.zeros((CAP, D), dtype=bf)
            xs[:cnt] = x[off:off + cnt].astype(bf)
            wu_p, wd_p = expert_weights(e)
            in_maps.append({
                "xp": _pack_x(xs.T), "wup": wu_p, "wdp": wd_p,
            })
        while len(in_maps) < N_CORES:  # idle cores in the last round
            if zero_map is None:
                zero_map = {
                    "xp": np.zeros((P, (D // P) * CAP), dtype=bf),
                    "wup": np.zeros((P, (H // P) * (D // P) * P), dtype=bf),
                    "wdp": np.zeros((P, (H // P) * D), dtype=bf),
                }
            in_maps.append(zero_map)

        res = run_bass_kernel_spmd(
            nc, in_maps, core_ids=list(range(N_CORES)), trace=_trace
        )
        LAST_RESULT = res
        for i, (e, off, cnt) in enumerate(round_slots):
            out[off:off + cnt] = res.results[i]["y"][:cnt].astype(x.dtype)
    return out


# revision 19
# speedup vs baseline: 1.1903x; 1.1903x over previous
"""Trainium2 Bass kernel: grouped MoE expert MLP (nn_ExpertGroup).

Strategy: expert parallelism across 8 NeuronCores. Tokens are sorted by
expert; core e runs expert e's two GEMMs:
    h = relu(x_e @ w_up[e].T) ** 2      (bf16, like the CUDA reference)
    y = h @ w_down[e].T
The host does the (free) token scatter/gather, the bf16 casts, and the
layout pre-packing so device DMAs read fat contiguous per-partition runs.

Host-packed HBM layouts (per core, cap = padded local token count):
    xp  [128, n_d*cap]     xp[pi, d*cap+t]         = x_e.T[d*128+pi, t]
    wup [128, n_j*n_d*128] wup[pi,(j*n_d+d)*128+c] = w_up[e].T[d*128+pi, j*128+c]
    wdp [128, n_j*D]       wdp[pi, j*D+i]          = w_down[e].T[j*128+pi, i]

Schedule (v5, driven by NTFF traces; the measured exec window runs from
the kernel-body start at ~6us to the LAST teardown instruction, so both
the DMA critical path at the head and the queue-drain tail count):
  - Each dma_start costs a flat ~0.6us trigger on the Sync queue, so the
    input is fetched with few triggers, ordered exactly as consumed.
  - GEMM1's first token chunk is 256 wide: the critical first bundle is
    only wu_j0 (0.25MB) + x[:, 0:256] (0.5MB), so real matmuls start
    ~3.5us earlier than with a 512-wide first chunk. w_up then streams
    per-j (0.25MB chunks): at 256-wide groups the PE consumes weights at
    ~290GB/s, just under the DMA rate, so per-j sems stay ahead.
  - PE warm-up dummies (no DMA deps) bridge preamble-end -> first data
    and keep the HAM clock ramp going (a PE gap costs ~2x: idle plus
    ~3us of 1.2GHz re-ramp, visible as 379ns matmuls in the trace).
  - relu runs on the Scalar (ACT) engine, the square on Vector: one op
    per engine per tile instead of two DVE ops.
  - PSUM start/stop accumulation is bank-granular: every accumulation
    group owns a whole [128, 512] PSUM tile (narrow groups just use a
    prefix of it). Never split one bank into independent regions.
  - Two tile pools (SBUF + PSUM) with per-tag bufs instead of five: the
    pool-exit barrier chain at the end of the measured window shrinks.
  - Output: one [128, 1024] DMA per token tile (both ic halves cast into
    one SBUF tile); the last tile runs ic0 + two 256-col groups in two
    PSUM banks so the first half's cast+DMA overlaps the second half's
    matmuls -> shorter serial tail.

Built on bacc.Bacc (not raw Bass): Bacc.compile() legalizes semaphore
waits to the TRN2 limit of one wait per instruction.
"""

import numpy as np
import ml_dtypes

import concourse.bass as bass
import concourse.mybir as mybir
import concourse.tile as tile
from concourse import bacc
from concourse.bass_utils import run_bass_kernel_spmd

T, D, H, E = 8192, 1024, 2048, 8
P = 128
N_CORES = 8
FD = 512   # matmul moving free dim (one PSUM bank of fp32)
C0 = 256   # width of the first (DMA-latency-critical) token chunk
# Dummy matmuls bridging the preamble->first-data window. The first input
# bundle lands 12.4-13.0us across runs (fixed DMA ring-start + sem latency,
# not descriptor-bound). Undershooting idles the PE and costs a ~1.4us HAM
# clock re-ramp (8 matmuls at 379ns instead of 216); overshooting only
# delays the stream by the overshoot. So size the warmup to end just PAST
# the p90 arrival (~13.4us): 70 dummies at ~56-107ns each.
WARMUP = 60


def _ensure_axon_ntff_hook():
    """The container's `antenv` stub lacks `axon_hooks`; if BASS_TRACE=1 is
    set, run_bass_kernel_spmd would crash importing it. Recreate the tiny
    registry and register the ctypes NTFF hook so tracing works (and never
    let this best-effort setup break the kernel)."""
    try:
        import antenv.axon_hooks  # noqa: F401
        return
    except ImportError:
        pass
    try:
        import sys
        import types

        import antenv
        from trn_agent_boot.trn_boot import _ntff_profile_via_ctypes

        mod = types.ModuleType("antenv.axon_hooks")
        mod._hook = _ntff_profile_via_ctypes("/opt/axon/libaxon_pjrt.so")
        mod.set_axon_ntff_profile_hook = lambda h: setattr(mod, "_hook", h)
        mod.get_axon_ntff_profile_hook = lambda: mod._hook
        sys.modules["antenv.axon_hooks"] = mod
        antenv.axon_hooks = mod
    except Exception:
        pass


_ensure_axon_ntff_hook()

_PROGRAM_CACHE: dict[int, "bass.Bass"] = {}
LAST_RESULT = None  # BassKernelResults of the most recent run (for harness use)


def _chunks(cap):
    """GEMM1 token chunks: first 512 split as 256+256 so the critical
    first DMA bundle is small; later chunks are full width."""
    n_tc = cap // FD
    return [(0, C0), (C0, FD)] + [(k * FD, (k + 1) * FD) for k in range(1, n_tc)]


def _build_program(cap: int) -> "bass.Bass":
    assert cap % FD == 0
    n_d = D // P       # 8  contraction tiles of GEMM1
    n_j = H // P       # 16 H partition tiles
    n_tc = cap // FD   # 512-wide token chunks
    n_t = cap // P     # token partition tiles (GEMM2 output)
    bf16 = mybir.dt.bfloat16
    f32 = mybir.dt.float32
    relu_fn = mybir.ActivationFunctionType.Relu

    nc = bacc.Bacc("TRN2", debug=False, num_devices=N_CORES)
    xp = nc.dram_tensor("xp", [P, n_d * cap], bf16, kind="ExternalInput")
    wup = nc.dram_tensor("wup", [P, n_j * n_d * P], bf16, kind="ExternalInput")
    wdp = nc.dram_tensor("wdp", [P, n_j * D], bf16, kind="ExternalInput")
    y = nc.dram_tensor("y", [cap, D], bf16, kind="ExternalOutput")

    wup4 = wup[:].rearrange("p (j d c) -> p j d c", j=n_j, d=n_d)
    wdp3 = wdp[:].rearrange("p (j f) -> p j f", j=n_j)          # [128, 16, D]
    y3 = y[:].rearrange("(po pi) f -> pi po f", pi=P)           # [128, n_t, D]

    chunks = _chunks(cap)

    with tile.TileContext(nc) as tc:
        with (
            tc.tile_pool(name="sb", bufs=1) as sb,
            tc.tile_pool(name="ps", bufs=7, space="PSUM") as psp,
        ):
            # x lives chunk-major in one flat SBUF region (matching the
            # host xp packing) so every x DMA is ONE contiguous run per
            # partition: 128 fat descriptors instead of 1024 small ones
            # (the strided form is descriptor-rate-bound, ~5us for 0.5MB).
            x_sb = sb.tile([P, n_d * cap], bf16)
            wuT_sb = sb.tile([P, n_j, n_d, P], bf16)
            wdT_sb = sb.tile([P, n_j, D], bf16)
            hsq_sb = sb.tile([P, n_j, cap], bf16)

            def xs(lo, hi, d):
                w = hi - lo
                return x_sb[:, n_d * lo + d * w: n_d * lo + (d + 1) * w]

            # Input DMAs, emitted first so the Sync queue leads with the
            # triggers (flat ~0.6us each), ordered exactly as consumed.
            # NOTE: do NOT prepend a tiny (sub-512B-per-partition) warm-up
            # DMA — runs with one consistently executed with the whole PE
            # clock ladder capped ~20% low (153us instead of 128us).
            lo0, hi0 = chunks[0]
            nc.sync.dma_start(out=wuT_sb[:, 0, :, :], in_=wup4[:, 0, :, :])
            nc.sync.dma_start(
                out=x_sb[:, n_d * lo0:n_d * hi0], in_=xp[:, n_d * lo0:n_d * hi0]
            )
            for j in range(1, n_j):
                nc.sync.dma_start(
                    out=wuT_sb[:, j, :, :], in_=wup4[:, j, :, :]
                )
            for lo, hi in chunks[1:]:
                nc.sync.dma_start(
                    out=x_sb[:, n_d * lo:n_d * hi], in_=xp[:, n_d * lo:n_d * hi]
                )
            for q in range(2):
                nc.sync.dma_start(
                    out=wdT_sb[:, q * 8:(q + 1) * 8, :],
                    in_=wdp3[:, q * 8:(q + 1) * 8, :],
                )

            # PE warm-up: dummy matmuls with no DMA dependencies bridge the
            # window from preamble end to first-data arrival and start the
            # HAM clock ramp early.
            warm = sb.tile([P, P], bf16)
            nc.vector.memset(warm[:], 0.0)
            wps = psp.tile([P, P], f32, tag="warm", bufs=1)
            for _ in range(WARMUP):
                nc.tensor.matmul(wps, warm[:], warm[:], start=True, stop=True)

            # GEMM1 + relu^2: hsq[j, t] = relu(h)^2 in [H, T] layout.
            for lo, hi in chunks:
                w = hi - lo
                for j in range(n_j):
                    ps = psp.tile([P, FD], f32, tag="ps")
                    for d in range(n_d):
                        nc.tensor.matmul(
                            ps[:, 0:w],
                            wuT_sb[:, j, d, :],
                            xs(lo, hi, d),
                            start=(d == 0),
                            stop=(d == n_d - 1),
                        )
                    hr = sb.tile([P, FD], bf16, tag="hr", bufs=4)
                    nc.scalar.activation(hr[:, 0:w], ps[:, 0:w], relu_fn)
                    nc.vector.tensor_mul(
                        out=hsq_sb[:, j, lo:hi], in0=hr[:, 0:w], in1=hr[:, 0:w]
                    )

            # GEMM2: y[t, i] = sum_j hsq[j, t].T @ wdT[j, i]
            n_ic = D // FD
            for t in range(n_t):
                yt = sb.tile([P, D], bf16, tag="yt", bufs=4)
                if t < n_t - 1:
                    # Both ic halves cast into one SBUF row tile -> one
                    # [128, 1024] out DMA per token tile.
                    for ic in range(n_ic):
                        ps = psp.tile([P, FD], f32, tag="ps")
                        for j in range(n_j):
                            nc.tensor.matmul(
                                ps,
                                hsq_sb[:, j, t * P:(t + 1) * P],
                                wdT_sb[:, j, ic * FD:(ic + 1) * FD],
                                start=(j == 0),
                                stop=(j == n_j - 1),
                            )
                        nc.vector.tensor_copy(
                            out=yt[:, ic * FD:(ic + 1) * FD], in_=ps
                        )
                    nc.sync.dma_start(out=y3[:, t, :], in_=yt)
                else:
                    ps = psp.tile([P, FD], f32, tag="ps")
                    for j in range(n_j):
                        nc.tensor.matmul(
                            ps,
                            hsq_sb[:, j, t * P:(t + 1) * P],
                            wdT_sb[:, j, 0:FD],
                            start=(j == 0),
                            stop=(j == n_j - 1),
                        )
                    nc.vector.tensor_copy(out=yt[:, 0:FD], in_=ps)
                    nc.sync.dma_start(out=y3[:, t, 0:FD], in_=yt[:, 0:FD])
                    # Final 512 cols: two 256-col groups in two PSUM banks;
                    # the first half's cast+DMA overlaps the second half's
                    # matmuls -> shorter serial tail.
                    for h in range(2):
                        hw = FD // 2
                        osl = slice(FD + h * hw, FD + (h + 1) * hw)
                        ph = psp.tile([P, FD], f32, tag="ps")
                        for j in range(n_j):
                            nc.tensor.matmul(
                                ph[:, 0:hw],
                                hsq_sb[:, j, t * P:(t + 1) * P],
                                wdT_sb[:, j, osl],
                                start=(j == 0),
                                stop=(j == n_j - 1),
                            )
                        nc.vector.tensor_copy(out=yt[:, osl], in_=ph[:, 0:hw])
                        nc.sync.dma_start(out=y3[:, t, osl], in_=yt[:, osl])

    nc.compile()
    return nc


def _get_program(cap: int) -> "bass.Bass":
    nc = _PROGRAM_CACHE.get(cap)
    if nc is None:
        nc = _build_program(cap)
        _PROGRAM_CACHE[cap] = nc
    return nc


CAP = 1024  # tokens per core per round (the uniform T/E split = one round)


def _pack_x(xs_T):
    """x_e.T (D, cap) -> xp [128, n_d*cap] chunk-major packed: for each
    token chunk [lo,hi), block[pi, n_d*lo + d*(hi-lo) + tl] =
    xs_T[d*128+pi, lo+tl]."""
    Dd, cap = xs_T.shape
    xt = xs_T.reshape(D // P, P, cap)
    blocks = [
        xt[:, :, lo:hi].transpose(1, 0, 2).reshape(P, (D // P) * (hi - lo))
        for lo, hi in _chunks(cap)
    ]
    return np.ascontiguousarray(np.concatenate(blocks, axis=1))


def _pack_wu(wuT):
    """w_up[e].T (D, H) -> wup [128, n_j*n_d*128]."""
    return np.ascontiguousarray(
        wuT.reshape(D // P, P, H // P, P)
        .transpose(1, 2, 0, 3)
        .reshape(P, (H // P) * (D // P) * P)
    )


def _pack_wd(wdT):
    """w_down[e].T (H, D) -> wdp [128, n_j*D]."""
    return np.ascontiguousarray(
        wdT.reshape(H // P, P, D).transpose(1, 0, 2).reshape(P, (H // P) * D)
    )


def kernel(x, num_tokens_per_expert, w_up, w_down, _trace=False):
    global LAST_RESULT
    bf = ml_dtypes.bfloat16
    x = np.asarray(x)
    counts = np.asarray(num_tokens_per_expert).astype(np.int64)
    w_up = np.asarray(w_up)
    w_down = np.asarray(w_down)
    n_tok = x.shape[0]
    assert counts.shape == (E,) and int(counts.sum()) == n_tok
    offsets = np.zeros(E, dtype=np.int64)
    offsets[1:] = np.cumsum(counts)[:-1]

    nc = _get_program(CAP)

    # Work list: split each expert's contiguous token segment into slots of
    # <= CAP tokens; process 8 slots per SPMD round. The uniform T/E = 1024
    # split is exactly one round of 8 slots.
    slots = []
    for e in range(E):
        cnt, off = int(counts[e]), int(offsets[e])
        for s in range(0, cnt, CAP):
            slots.append((e, off + s, min(CAP, cnt - s)))

    wu_cache = {}
    wd_cache = {}

    def expert_weights(e):
        if e not in wu_cache:
            wu_cache[e] = _pack_wu(w_up[e].astype(bf).T)
            wd_cache[e] = _pack_wd(w_down[e].astype(bf).T)
        return wu_cache[e], wd_cache[e]

    out = np.zeros((n_tok, D), dtype=x.dtype)
    zero_map = None
    for r0 in range(0, len(slots), N_CORES):
        round_slots = slots[r0:r0 + N_CORES]
        in_maps = []
        for e, off, cnt in round_slots:
            xs = np.zeros((CAP, D), dtype=bf)
            xs[:cnt] = x[off:off + cnt].astype(bf)
            wu_p, wd_p = expert_weights(e)
            in_maps.append({
                "xp": _pack_x(xs.T), "wup": wu_p, "wdp": wd_p,
            })
        while len(in_maps) < N_CORES:  # idle cores in the last round
            if zero_map is None:
                zero_map = {
                    "xp": np.zeros((P, (D // P) * CAP), dtype=bf),
                    "wup": np.zeros((P, (H // P) * (D // P) * P), dtype=bf),
                    "wdp": np.zeros((P, (H // P) * D), dtype=bf),
                }
            in_maps.append(zero_map)

        res = run_bass_kernel_spmd(
            nc, in_maps, core_ids=list(range(N_CORES)), trace=_trace
        )
        LAST_RESULT = res
        for i, (e, off, cnt) in enumerate(round_slots):
            out[off:off + cnt] = res.results[i]["y"][:cnt].astype(x.dtype)
    return out
